# revision 21
# baseline (speedup 1.0000x reference)
"""BoundaryLoss kernel for 8 Trainium2 NeuronCores.

Math (equivalent to the reference):
  boundary(i,j) = [L(i,j+1) != L(i,j-1)]_edge OR [L(i+1,j) != L(i-1,j)]_edge
    (the union of class-1/class-2 indicator boundaries equals "any label
     change" because any differing pair in {0,1,2} differs in membership
     of class 1 or class 2; |gx|+|gy| > 0.1 iff either diff is nonzero)
  ce = logsumexp_c(x) - x[label]        (max-free: |x| <= ~6 so exp is safe)
  loss = sum(ce * boundary) / (sum(boundary) + 1e-8)

Sharding: pure data parallel, 4 images per core.  Each core writes
per-column partial sums of (boundary, ce*boundary); the host sums the
8 * [1, 1024] partials in float64 and does the final division.

v2 design (vs the first working version):
  - labels arrive in SBUF already cast to bf16 via SWDGE (gpsimd) DMA
    dtype-cast; seam rows (partition-crossing row neighbours for the
    vertical gradient) are re-read straight from HBM instead of
    partition-shifted SBUF->SBUF copies (which serialized on one DMA
    queue).
  - x[label] is selected in the x domain with copy_predicated
    (select()) instead of exp-domain masked products + a second Ln:
    drops one activation pass and two DVE passes per chunk.
  - the m2 mask is computed on the scalar engine as Relu(L-1) to
    offload the vector engine (the overall bottleneck).
  - per-pixel products reduce on the tensor engine (matmul with a ones
    vector) which is otherwise idle.
"""

import numpy as np

B, C, H, W = 32, 3, 768, 768
NCORES = 8
BLOC = B // NCORES  # images per core
P = 128
TPB = H // P        # rows per partition (6)
NH = 2              # chunks (halves) per image
RPC = TPB // NH     # rows per chunk (3)
CHW = RPC * W       # columns per chunk (2304)

_CACHE = {}


def _build(label_words):
    """Build + compile the Bass module. label_words = int32 words per label
    element (2 for int64 inputs, 1 for int32)."""
    import concourse.bacc as bacc
    import concourse.tile as tile
    import concourse.mybir as mybir
    from bass_rust import add_dep_helper as _add_dep

    fp32 = mybir.dt.float32
    bf16 = mybir.dt.bfloat16
    i32 = mybir.dt.int32
    Alu = mybir.AluOpType
    Act = mybir.ActivationFunctionType

    nc = bacc.Bacc(
        "TRN2",
        target_bir_lowering=False,
        debug=False,
        enable_asserts=False,
        num_devices=NCORES,
    )
    preds = nc.dram_tensor(
        "preds", [BLOC, C, P, TPB * W], fp32, kind="ExternalInput"
    ).ap()
    labs = nc.dram_tensor(
        "labs", [BLOC, P, TPB * W, label_words], i32, kind="ExternalInput"
    ).ap()
    outp = nc.dram_tensor("partials", [1, 1024], fp32, kind="ExternalOutput").ap()

    with tile.TileContext(nc) as tc:
        with (
            tc.tile_pool(name="ps", bufs=1, space="PSUM") as ps_pool,
            tc.tile_pool(name="lab", bufs=2) as lab_pool,
            tc.tile_pool(name="xin", bufs=2) as x_pool,
            tc.tile_pool(name="eact", bufs=2) as e_pool,
            tc.tile_pool(name="sls", bufs=2) as s_pool,
            tc.tile_pool(name="wrk", bufs=1) as wrk,
            tc.tile_pool(name="xch", bufs=2) as xch_pool,
            tc.tile_pool(name="accp", bufs=1) as accp,
        ):
            ones = accp.tile([P, 1], bf16, name="ones")
            nc.vector.memset(ones[:], 1.0)
            negone = accp.tile([P, 1], fp32, name="negone")
            nc.vector.memset(negone[:], -1.0)
            pb = ps_pool.tile([1, 512], fp32, name="pb")
            pcb = ps_pool.tile([1, 512], fp32, name="pcb")
            SLABS = [(0, 512), (512, 1024), (1024, 1536), (1536, 2048), (2048, 2304)]
            for b in range(BLOC):
                # Lb rows: [U, r0..r5, D]; U[p] = image row 6p-1 (clamped at
                # top), D[p] = image row 6p+6 (clamped at bottom).  All loads
                # are plain affine HBM reads; the SWDGE path casts i32->bf16
                # in the DMA datapath.
                # All label loads ride the fast pipelined HWDGE queue as i32
                # (the SWDGE cast-DMA queue is serial, ~6.5us per dispatch,
                # and became the kernel's critical path).  The i32 -> bf16
                # cast runs on the otherwise-idle GpSimd engine.  One tile
                # per producer DMA so readers wait on exactly the transfer
                # they need.
                Us = lab_pool.tile([P, W], i32, name="Us", tag="Us")
                nc.sync.dma_start(
                    out=Us[1:P, :],
                    in_=labs[b, 0 : P - 1, (TPB - 1) * W : TPB * W, 0:1],
                )
                nc.sync.dma_start(out=Us[0:1, :], in_=labs[b, 0:1, 0:W, 0:1])
                Ds = lab_pool.tile([P, W], i32, name="Ds", tag="Ds")
                nc.sync.dma_start(
                    out=Ds[0 : P - 1, :], in_=labs[b, 1:P, 0:W, 0:1]
                )
                nc.sync.dma_start(
                    out=Ds[P - 1 : P, :],
                    in_=labs[b, P - 1 : P, (TPB - 1) * W : TPB * W, 0:1],
                )
                Lm = []
                for h in range(NH):
                    lmi = lab_pool.tile(
                        [P, RPC, W], i32, name=f"Lmi{h}", tag="Lmi"
                    )
                    nc.sync.dma_start(
                        out=lmi[:],
                        in_=labs[b, :, h * RPC * W : (h + 1) * RPC * W, 0:1],
                    )
                    lmc = lab_pool.tile(
                        [P, RPC, W], bf16, name=f"Lm{h}", tag=f"Lm{h}"
                    )
                    nc.gpsimd.tensor_copy(lmc[:], lmi[:])
                    Lm.append(lmc)
                for h in range(NH):
                    Lr = Lm[h][:]  # chunk label rows [P, RPC, W]
                    xs = []
                    for ch in range(C):
                        x = x_pool.tile([P, CHW], fp32, name=f"x{ch}", tag=f"x{ch}")
                        nc.sync.dma_start(
                            out=x[:],
                            in_=preds[b, ch, :, h * CHW : (h + 1) * CHW],
                        )
                        xs.append(x)
                    # --- logsumexp numerator --------------------------------
                    es = []
                    for ch in range(C):
                        e = e_pool.tile([P, CHW], bf16, name=f"e{ch}", tag=f"e{ch}")
                        nc.scalar.activation(e[:], xs[ch][:], Act.Exp)
                        es.append(e)
                    s1 = wrk.tile([P, CHW], bf16, name="s1", tag="s1")
                    nc.vector.tensor_add(s1[:], es[0][:], es[1][:])
                    s2 = s_pool.tile([P, CHW], bf16, name="s2", tag="s2")
                    nc.vector.tensor_add(s2[:], s1[:], es[2][:])
                    lse = s_pool.tile([P, CHW], bf16, name="lse", tag="lse")
                    nc.scalar.activation(lse[:], s2[:], Act.Ln)

                    # --- x[label] via predicated overwrite ------------------
                    m1 = wrk.tile([P, RPC, W], bf16, name="m1", tag="m1")
                    nc.vector.tensor_scalar(m1[:], Lr, 1.0, None, Alu.is_equal)
                    m2 = s_pool.tile([P, RPC, W], bf16, name="m2", tag="m2")
                    nc.scalar.activation(m2[:], Lr, Act.Relu, bias=negone[:])
                    xsel = s_pool.tile([P, CHW], bf16, name="xsel", tag="xsel")
                    nc.scalar.activation(xsel[:], xs[0][:], Act.Copy)
                    # CopyPredicated wants an integer mask; bf16 0.0/1.0
                    # bitcast to int16 is 0 / 0x3F80 — same truthiness.
                    nc.vector.copy_predicated(
                        xsel[:], m1[:].bitcast(mybir.dt.int16), xs[1][:]
                    )
                    i_cp2 = nc.vector.copy_predicated(
                        xsel[:], m2[:].bitcast(mybir.dt.int16), xs[2][:]
                    )

                    # --- boundary mask --------------------------------------
                    nx = wrk.tile([P, RPC, W], bf16, name="nx", tag="nx")
                    nc.vector.tensor_tensor(
                        nx[:, :, 1 : W - 1],
                        Lr[:, :, 0 : W - 2],
                        Lr[:, :, 2:W],
                        Alu.not_equal,
                    )
                    nc.vector.tensor_tensor(
                        nx[:, :, 0:1], Lr[:, :, 0:1], Lr[:, :, 1:2], Alu.not_equal
                    )
                    nc.vector.tensor_tensor(
                        nx[:, :, W - 1 : W],
                        Lr[:, :, W - 2 : W - 1],
                        Lr[:, :, W - 1 : W],
                        Alu.not_equal,
                    )
                    # ny row r compares image rows r-1 and r+1; rows live in
                    # (Us | Lm0 | Lm1 | Ds) tiles, so emit one inst per row
                    # with exactly the producers it needs.
                    ny = wrk.tile([P, RPC, W], bf16, name="ny", tag="ny")
                    if h == 0:
                        pairs = [
                            (Us[:], Lm[0][:, 1, :]),
                            (Lm[0][:, 0, :], Lm[0][:, 2, :]),
                            (Lm[0][:, 1, :], Lm[1][:, 0, :]),
                        ]
                    else:
                        pairs = [
                            (Lm[0][:, 2, :], Lm[1][:, 1, :]),
                            (Lm[1][:, 0, :], Lm[1][:, 2, :]),
                            (Lm[1][:, 1, :], Ds[:]),
                        ]
                    # Order-only edges: keep the late-arriving-label ny rows
                    # behind the predictions-dependent chain so they cannot
                    # head-block the in-order vector stream during warmup.
                    for j, (top, bot) in enumerate(pairs):
                        i_ny = nc.vector.tensor_tensor(
                            ny[:, j, :], top, bot, Alu.not_equal
                        )
                        _add_dep(i_ny.ins, i_cp2.ins, sync=False,
                                 reason="schedule ny after CP chain")
                    bnd = wrk.tile([P, CHW], bf16, name="bnd", tag="bnd")
                    nc.vector.tensor_tensor(bnd[:], nx[:], ny[:], Alu.max)

                    # --- weighted CE and reductions -------------------------
                    ce = wrk.tile([P, CHW], bf16, name="ce", tag="ce")
                    nc.vector.tensor_sub(ce[:], lse[:], xsel[:])
                    cb = wrk.tile([P, CHW], bf16, name="cb", tag="cb")
                    nc.vector.tensor_mul(cb[:], ce[:], bnd[:])

                    first = b == 0 and h == 0
                    last = b == BLOC - 1 and h == NH - 1
                    for k, (a0, a1) in enumerate(SLABS):
                        nc.tensor.matmul(
                            pb[:, 0 : a1 - a0],
                            ones[:],
                            bnd[:, a0:a1],
                            start=first and k == 0,
                            stop=last and k == len(SLABS) - 1,
                        )
                        nc.tensor.matmul(
                            pcb[:, 0 : a1 - a0],
                            ones[:],
                            cb[:, a0:a1],
                            start=first and k == 0,
                            stop=last and k == len(SLABS) - 1,
                        )
            sb = accp.tile([1, 1024], fp32, name="sb")
            nc.vector.tensor_copy(sb[:, 0:512], pb[:, :])
            nc.vector.tensor_copy(sb[:, 512:1024], pcb[:, :])
            nc.sync.dma_start(out=outp[:, :], in_=sb[:])

    # Pin Exp/Ln/Copy/Relu to the one table set containing all of them so the
    # ACT table loads once instead of thrashing between sets.
    from concourse import hw_specs

    KEEP = "natural_log_exp_and_others"
    orig = hw_specs.get_activation_tables

    def only_combined(arch):
        t = orig(arch)
        return {name: (funcs if name == KEEP else set()) for name, funcs in t.items()}

    patched = []
    for mod in (hw_specs, bacc):
        if getattr(mod, "get_activation_tables", None) is not None:
            patched.append((mod, mod.get_activation_tables))
            mod.get_activation_tables = only_combined
    try:
        nc.compile()
    finally:
        for mod, fn in patched:
            mod.get_activation_tables = fn
    return nc


def _get_nc(label_words):
    if label_words not in _CACHE:
        _CACHE[label_words] = _build(label_words)
    return _CACHE[label_words]


def kernel(predictions, labels):
    from concourse.bass_utils import run_bass_kernel_spmd

    preds = np.ascontiguousarray(predictions, dtype=np.float32).reshape(
        NCORES, BLOC, C, P, TPB * W
    )
    labels = np.ascontiguousarray(labels)
    if labels.dtype == np.int64:
        label_words = 2
        labs32 = labels.view("<i4")
    elif labels.dtype == np.int32:
        label_words = 1
        labs32 = labels.reshape(labels.shape + (1,))
    else:
        raise ValueError(f"unsupported labels dtype {labels.dtype}")
    labs32 = labs32.reshape(NCORES, BLOC, P, TPB * W, label_words)

    nc = _get_nc(label_words)
    in_maps = [
        {"preds": preds[i], "labs": labs32[i]} for i in range(NCORES)
    ]
    res = run_bass_kernel_spmd(nc, in_maps, list(range(NCORES))).results
    tot_b = 0.0
    tot_cb = 0.0
    for r in res:
        p = r["partials"].astype(np.float64)
        tot_b += p[0, :512].sum()
        tot_cb += p[0, 512:].sum()
    return np.float32(tot_cb / (tot_b + 1e-8))
